# revision 14
# baseline (speedup 1.0000x reference)
"""Trainium2 Bass kernel for MetabolicAttentionHead.

Contract: kernel(**inputs) takes FULL inputs (x [4,4096,1024] f32, Wq/Wk/Wv
[64,1024] f32, energy scalar f32) and returns the FULL reference output tuple
(out [4,4096,64] f32, attn [4,4096,4096] f32, new_energy scalar f32).

Sharding: 8 cores; core c handles batch b = c//2 and query half qh = c%2
(2048 query rows). Host pre-transposes x[b] to xT [1024, 4096] per core, with
the core's query half rolled to the front so the SPMD program always reads
queries from columns 0:2048. The k-order roll is undone at host gather time.

Device per core:
  phase 1: qT/kT/vT projections (fp32r, M=64 stationaries), PE-transposes of
           v/k chunks into vk [128k, v|k], interleaved with the block loop.
  phase 2: scores[q,k] = qT'.kT (fp32r, M=128 2-pass), exp on ACT with
           accum -> Z, normalize on DVE, DMA attn rows out.
  phase 3 (bf16): scoresT = kT_bf.qT_bf (N=1024), exp -> es bf16,
           outG += vk_bf^T . es over all k: rows 0:64 = out^T (unnormalized),
           rows 64:128 = G[h,q] for the entropy identity
           sum_k a*s = (sum_h qT'[h,q] G[h,q]) / Z.
  phase 4: PE-transpose out^T -> out rows, scale by 1/Z, ship out/Z/qT/G.

Host: entropy E_q = ln Z_q - P_q/Z_q - T*eps  (P = sum_h qT'*G, eps = 1e-8),
new_energy = 0.9*energy + 0.1*mean(E).
"""

import os

os.environ.pop("JAX_PLATFORMS", None)

import numpy as np

import concourse.bass as bass
import concourse.bacc as bacc
import concourse.tile as tile
from concourse import mybir
from concourse import bass_utils

B, T, C, H = 4, 4096, 1024, 64
Q = T // 2  # queries per core
GAMMA = 1.0 / np.sqrt(H).astype(np.float32)
N_CORES = 8



def build_nc():
    nc = bacc.Bacc("TRN2", target_bir_lowering=False)
    f32 = mybir.dt.float32
    fp16 = mybir.dt.float16

    xt = nc.dram_tensor("xt", [C, T], fp16, kind="ExternalInput")
    wq = nc.dram_tensor("wq", [C, H], f32, kind="ExternalInput")
    wkv = nc.dram_tensor("wkv", [C, 128], f32, kind="ExternalInput")
    id64 = nc.dram_tensor("id64", [H, H], f32, kind="ExternalInput")

    attn_d = nc.dram_tensor("attn", [Q, T], f32, kind="ExternalOutput")
    outp_d = nc.dram_tensor("outp", [Q, H], f32, kind="ExternalOutput")
    zt_d = nc.dram_tensor("zt", [128, Q // 128], f32, kind="ExternalOutput")
    qt_d = nc.dram_tensor("qt", [H, Q], fp16, kind="ExternalOutput")
    gt_d = nc.dram_tensor("gt", [H, Q], f32, kind="ExternalOutput")

    NQT = Q // 128  # 16 q-tiles
    NKT = T // 128  # 32 k-tiles

    with tile.TileContext(nc) as tc:
        with (
            tc.tile_pool(name="consts", bufs=1) as consts,
            tc.tile_pool(name="big", bufs=1) as big,
        ):
            wq_sb = consts.tile([128, 8, H], fp16, tag="wq")
            wkv_sb = consts.tile([128, 8, 128], fp16, tag="wkv")
            id_lo = consts.tile([H, H], fp16, tag="id_lo")
            id_hi = consts.tile([128, H], fp16, tag="id_hi")
            id_f32 = consts.tile([H, H], f32, tag="id_f32")
            nc.gpsimd.dma_start(out=wq_sb, in_=wq.rearrange("(cc p) h -> p cc h", p=128))
            nc.gpsimd.dma_start(out=wkv_sb, in_=wkv.rearrange("(cc p) h -> p cc h", p=128))
            nc.gpsimd.dma_start(out=id_lo, in_=id64[:, :])
            nc.gpsimd.dma_start(out=id_hi[H:128, :], in_=id64[:, :])
            nc.sync.dma_start(out=id_f32, in_=id64[:, :])

            qk = big.tile([H, Q + T], fp16, tag="qk")  # rows 0:64: qT | kT
            vk = big.tile([128, NKT, 128], fp16, tag="vk")  # [:,j,0:64]=v, 64:=k
            z_sb = big.tile([128, NQT], f32, tag="z")
            rz_sb = big.tile([128, NQT], f32, tag="rz")
            outT_sb = big.tile([H, Q], f32, tag="outT")
            g_sb = big.tile([128, Q], f32, tag="g")
            outn_sb = big.tile([128, NQT, H], f32, tag="outn")

            qT = qk[0:H, 0:Q]
            kT = qk[0:H, Q : Q + T]

            # ---------------- phase 1: projections + v/k transposes ----------
            with (
                tc.tile_pool(name="ph1", bufs=2) as ph1,
                tc.tile_pool(name="ps_proj", bufs=2, space="PSUM") as ps_proj,
                tc.tile_pool(name="ps_tr", bufs=1, space="PSUM") as ps_tr,
            ):
                vT_hi = big.tile([128, T], fp16, tag="vT")  # rows 64:128 used
                xt_r = xt.rearrange("(cc p) t -> p cc t", p=128)
                for j in range(8):  # t-blocks of 512
                    xb = ph1.tile([128, 8, 512], fp16, tag="xb")
                    nc.sync.dma_start(out=xb, in_=xt_r[:, :, j * 512 : (j + 1) * 512])
                    sl = slice(j * 512, (j + 1) * 512)
                    ps = ps_proj.tile([128, 512], f32, tag="ps_kv")
                    for cc in range(8):
                        nc.tensor.matmul(
                            ps, wkv_sb[:, cc, :], xb[:, cc, :],
                            start=(cc == 0), stop=(cc == 7),
                        )
                    nc.vector.tensor_copy(kT[:, sl], ps[0:H, :])
                    nc.vector.tensor_copy(vT_hi[H:128, sl], ps[H:128, :])
                    if j < 4:
                        psq = ps_proj.tile([H, 512], f32, tag="ps_q")
                        for cc in range(8):
                            nc.tensor.matmul(
                                psq, wq_sb[:, cc, :], xb[:, cc, :],
                                start=(cc == 0), stop=(cc == 7),
                            )
                        nc.vector.tensor_copy(qT[:, sl], psq)
                    # transposes for this block's 4 128-chunks
                    for t in range(4):
                        jj = j * 4 + t
                        c0 = jj * 128
                        ptv = ps_tr.tile([128, H], fp16, tag="ptv")
                        nc.tensor.transpose(ptv, vT_hi[H:128, c0 : c0 + 128], id_hi[H:128, :])
                        nc.vector.tensor_copy(vk[:, jj, 0:H], ptv)
                        ptk = ps_tr.tile([128, H], fp16, tag="ptk")
                        nc.tensor.transpose(ptk, kT[:, c0 : c0 + 128], id_lo)
                        nc.vector.tensor_copy(vk[:, jj, H:128], ptk)

            # ------- merged phases 2+3: scores/softmax/attn + scoresT/out/G --
            # Per q-tile: scores + exp1 for both k-halves, with 4 phase-3
            # j-steps interleaved to keep PE dense (HAM warm) while ACT runs.
            # oG accumulates q-half 0 during q-tiles 0:8, q-half 1 during 8:16.
            with (
                tc.tile_pool(name="attn_t", bufs=3) as attn_pool,
                tc.tile_pool(name="zh", bufs=4) as zh_pool,
                tc.tile_pool(name="est", bufs=3) as est_pool,
                tc.tile_pool(name="ps_s", bufs=1, space="PSUM") as ps_s,
                tc.tile_pool(name="ps_og", bufs=1, space="PSUM") as ps_og,
                tc.tile_pool(name="ps_st", bufs=1, space="PSUM") as ps_st,
            ):
                def p3_step(half, j):
                    st = ps_st.tile([128, 1024], f32, tag="st")
                    for qb in range(2):
                        osl = slice(qb * 512, (qb + 1) * 512)
                        nc.tensor.matmul(
                            st[:, osl],
                            qk[0:H, Q + j * 128 : Q + (j + 1) * 128],
                            qk[0:H, half * 1024 + qb * 512 : half * 1024 + (qb + 1) * 512],
                            start=True, stop=True,
                        )
                    es = est_pool.tile([128, 1024], fp16, tag="es")
                    nc.scalar.activation(out=es, in_=st, func=mybir.ActivationFunctionType.Exp)
                    for qb in range(2):
                        osl = slice(qb * 512, (qb + 1) * 512)
                        nc.tensor.matmul(
                            oG[:, osl],
                            vk[:, j, :],
                            es[:, osl],
                            start=(j == 0), stop=(j == NKT - 1),
                            skip_group_check=True,
                        )

                oG = None
                for i in range(NQT):
                    p3h = i // (NQT // 2)  # oG half for this q-tile
                    if i % (NQT // 2) == 0:
                        oG = ps_og.tile([128, 1024], f32, tag="oG")
                    at = attn_pool.tile([128, T], f32, tag="at")
                    zh = zh_pool.tile([128, 2], f32, tag="zh")
                    for half in range(2):
                        ps = ps_s.tile([128, 2048], f32, tag="ps_s")
                        for kb in range(4):
                            nc.tensor.matmul(
                                ps[:, kb * 512 : (kb + 1) * 512],
                                qk[0:H, i * 128 : (i + 1) * 128],
                                qk[0:H, Q + half * 2048 + kb * 512 : Q + half * 2048 + (kb + 1) * 512],
                                start=True, stop=True,
                            )
                        nc.scalar.activation(
                            out=at[:, half * 2048 : (half + 1) * 2048], in_=ps,
                            func=mybir.ActivationFunctionType.Exp,
                            accum_out=zh[:, half : half + 1],
                        )
                        for s in range(2):
                            p3_step(p3h, (i % (NQT // 2)) * 4 + half * 2 + s)
                    nc.vector.tensor_add(z_sb[:, i : i + 1], zh[:, 0:1], zh[:, 1:2])
                    nc.vector.reciprocal(rz_sb[:, i : i + 1], z_sb[:, i : i + 1])
                    nc.vector.tensor_scalar_mul(at, at, rz_sb[:, i : i + 1])
                    nc.sync.dma_start(out=attn_d[i * 128 : (i + 1) * 128, :], in_=at)
                    if i % (NQT // 2) == (NQT // 2) - 1:
                        qsl = slice(p3h * 1024, (p3h + 1) * 1024)
                        nc.vector.tensor_copy(outT_sb[:, qsl], oG[0:H, :])
                        nc.vector.tensor_copy(g_sb[H:128, qsl], oG[H:128, :])

            # ---------------- phase 4: finalize ------------------------------
            nc.sync.dma_start(out=gt_d[:, :], in_=g_sb[H:128, :])
            nc.sync.dma_start(out=qt_d[:, :], in_=qT)
            nc.sync.dma_start(out=zt_d[:, :], in_=z_sb)
            with tc.tile_pool(name="ps_ot", bufs=4, space="PSUM") as ps_ot:
                for jj in range(NQT):
                    po = ps_ot.tile([128, H], f32, tag="po")
                    nc.tensor.transpose(po, outT_sb[:, jj * 128 : (jj + 1) * 128], id_f32)
                    nc.vector.tensor_scalar_mul(outn_sb[:, jj, :], po, rz_sb[:, jj : jj + 1])
            nc.sync.dma_start(out=outp_d.rearrange("(jj p) h -> p jj h", p=128), in_=outn_sb)

    nc.compile()
    return nc


_NC_CACHE = None


def _get_nc():
    global _NC_CACHE
    if _NC_CACHE is None:
        _NC_CACHE = build_nc()
    return _NC_CACHE


def make_in_maps(x, Wq, Wk, Wv):
    id64 = np.eye(H, dtype=np.float32)
    wq_h = np.ascontiguousarray(Wq.T).astype(np.float32) * GAMMA
    wkv_h = np.ascontiguousarray(np.concatenate([Wk.T, Wv.T], axis=1)).astype(np.float32)
    in_maps = []
    xts = {}
    for b in range(B):
        xts[b] = x[b].T.astype(np.float16)  # [C, T] fp16 (device rounds anyway)
    for c in range(N_CORES):
        b, qh = c // 2, c % 2
        xt = xts[b]
        if qh == 1:
            xt = np.ascontiguousarray(
                np.concatenate([xt[:, Q:], xt[:, :Q]], axis=1)
            )
        else:
            xt = np.ascontiguousarray(xt)
        in_maps.append({"xt": xt, "wq": wq_h, "wkv": wkv_h, "id64": id64})
    return in_maps


def gather_outputs(results, energy):
    out = np.empty((B, T, H), dtype=np.float32)
    attn = np.empty((B, T, T), dtype=np.float32)
    ent_sum = 0.0
    for c in range(N_CORES):
        b, qh = c // 2, c % 2
        r = results[c]
        qsl = slice(qh * Q, (qh + 1) * Q)
        out[b, qsl, :] = r["outp"]
        a = r["attn"]
        if qh == 0:
            attn[b, qsl, :] = a
        else:
            attn[b, qsl, Q:] = a[:, :Q]
            attn[b, qsl, :Q] = a[:, Q:]
        Zq = r["zt"].T.reshape(Q).astype(np.float64)  # q = i*128 + p
        P = np.einsum("hq,hq->q", r["qt"].astype(np.float64), r["gt"].astype(np.float64))
        Eq = np.log(Zq) - P / Zq - T * 1e-8
        ent_sum += Eq.sum()
    ent_mean = np.float32(ent_sum / (B * T))
    new_energy = np.float32(0.9 * np.float32(energy) + 0.1 * ent_mean)
    return out, attn, new_energy


def run_on_hw(in_maps, trace=False):
    nc = _get_nc()
    return bass_utils.run_bass_kernel_spmd(
        nc, in_maps, core_ids=list(range(N_CORES)), trace=trace
    )


def kernel(x, Wq, Wk, Wv, energy):
    in_maps = make_in_maps(
        np.asarray(x, dtype=np.float32),
        np.asarray(Wq, dtype=np.float32),
        np.asarray(Wk, dtype=np.float32),
        np.asarray(Wv, dtype=np.float32),
    )
    res = run_on_hw(in_maps, trace=False)
    return gather_outputs(res.results, np.asarray(energy, dtype=np.float32))


# revision 15
# speedup vs baseline: 1.2935x; 1.2935x over previous
"""Trainium2 Bass kernel for MetabolicAttentionHead.

Contract: kernel(**inputs) takes FULL inputs (x [4,4096,1024] f32, Wq/Wk/Wv
[64,1024] f32, energy scalar f32) and returns the FULL reference output tuple
(out [4,4096,64] f32, attn [4,4096,4096] f32, new_energy scalar f32).

Sharding: 8 cores; core c handles batch b = c//2 and query half qh = c%2
(2048 query rows). Host pre-transposes x[b] to xT [1024, 4096] per core, with
the core's query half rolled to the front so the SPMD program always reads
queries from columns 0:2048. The k-order roll is undone at host gather time.

Device per core:
  phase 1: qT/kT/vT projections (fp32r, M=64 stationaries), PE-transposes of
           v/k chunks into vk [128k, v|k], interleaved with the block loop.
  phase 2: scores[q,k] = qT'.kT (fp32r, M=128 2-pass), exp on ACT with
           accum -> Z, normalize on DVE, DMA attn rows out.
  phase 3 (bf16): scoresT = kT_bf.qT_bf (N=1024), exp -> es bf16,
           outG += vk_bf^T . es over all k: rows 0:64 = out^T (unnormalized),
           rows 64:128 = G[h,q] for the entropy identity
           sum_k a*s = (sum_h qT'[h,q] G[h,q]) / Z.
  phase 4: PE-transpose out^T -> out rows, scale by 1/Z, ship out/Z/qT/G.

Host: entropy E_q = ln Z_q - P_q/Z_q - T*eps  (P = sum_h qT'*G, eps = 1e-8),
new_energy = 0.9*energy + 0.1*mean(E).
"""

import os

os.environ.pop("JAX_PLATFORMS", None)

import numpy as np

import concourse.bass as bass
import concourse.bacc as bacc
import concourse.tile as tile
from concourse import mybir
from concourse import bass_utils

B, T, C, H = 4, 4096, 1024, 64
Q = T // 2  # queries per core
GAMMA = 1.0 / np.sqrt(H).astype(np.float32)
N_CORES = 8



def build_nc():
    nc = bacc.Bacc("TRN2", target_bir_lowering=False)
    f32 = mybir.dt.float32
    fp16 = mybir.dt.float16

    xt = nc.dram_tensor("xt", [C, T], fp16, kind="ExternalInput")
    wq = nc.dram_tensor("wq", [C, H], f32, kind="ExternalInput")
    wkv = nc.dram_tensor("wkv", [C, 128], f32, kind="ExternalInput")
    id64 = nc.dram_tensor("id64", [H, H], f32, kind="ExternalInput")

    attn_d = nc.dram_tensor("attn", [Q, T], f32, kind="ExternalOutput")
    outp_d = nc.dram_tensor("outp", [Q, H], f32, kind="ExternalOutput")
    zt_d = nc.dram_tensor("zt", [128, Q // 128], f32, kind="ExternalOutput")
    qt_d = nc.dram_tensor("qt", [H, Q], fp16, kind="ExternalOutput")
    gt_d = nc.dram_tensor("gt", [H, Q], f32, kind="ExternalOutput")

    NQT = Q // 128  # 16 q-tiles
    NKT = T // 128  # 32 k-tiles

    with tile.TileContext(nc) as tc:
        with (
            tc.tile_pool(name="consts", bufs=1) as consts,
            tc.tile_pool(name="big", bufs=1) as big,
        ):
            wq_sb = consts.tile([128, 8, H], fp16, tag="wq")
            wkv_sb = consts.tile([128, 8, 128], fp16, tag="wkv")
            id_lo = consts.tile([H, H], fp16, tag="id_lo")
            id_hi = consts.tile([128, H], fp16, tag="id_hi")
            id_f32 = consts.tile([H, H], f32, tag="id_f32")
            nc.gpsimd.dma_start(out=wq_sb, in_=wq.rearrange("(cc p) h -> p cc h", p=128))
            nc.gpsimd.dma_start(out=wkv_sb, in_=wkv.rearrange("(cc p) h -> p cc h", p=128))
            nc.gpsimd.dma_start(out=id_lo, in_=id64[:, :])
            nc.gpsimd.dma_start(out=id_hi[H:128, :], in_=id64[:, :])
            nc.sync.dma_start(out=id_f32, in_=id64[:, :])

            qk = big.tile([H, Q + T], fp16, tag="qk")  # rows 0:64: qT | kT
            vk = big.tile([128, NKT, 128], fp16, tag="vk")  # [:,j,0:64]=v, 64:=k
            z_sb = big.tile([128, NQT], f32, tag="z")
            rz_sb = big.tile([128, NQT], f32, tag="rz")
            outT_sb = big.tile([H, Q], f32, tag="outT")
            g_sb = big.tile([128, Q], f32, tag="g")
            outn_sb = big.tile([128, NQT, H], f32, tag="outn")

            qT = qk[0:H, 0:Q]
            kT = qk[0:H, Q : Q + T]

            # ---------------- phase 1: projections + v/k transposes ----------
            with (
                tc.tile_pool(name="ph1", bufs=2) as ph1,
                tc.tile_pool(name="ps_proj", bufs=2, space="PSUM") as ps_proj,
                tc.tile_pool(name="ps_tr", bufs=1, space="PSUM") as ps_tr,
            ):
                vT_hi = big.tile([128, T], fp16, tag="vT")  # rows 64:128 used
                xt_r = xt.rearrange("(cc p) t -> p cc t", p=128)
                for j in range(8):  # t-blocks of 512
                    xb = ph1.tile([128, 8, 512], fp16, tag="xb")
                    nc.sync.dma_start(out=xb, in_=xt_r[:, :, j * 512 : (j + 1) * 512])
                    sl = slice(j * 512, (j + 1) * 512)
                    ps = ps_proj.tile([128, 512], f32, tag="ps_kv")
                    for cc in range(8):
                        nc.tensor.matmul(
                            ps, wkv_sb[:, cc, :], xb[:, cc, :],
                            start=(cc == 0), stop=(cc == 7),
                        )
                    nc.vector.tensor_copy(kT[:, sl], ps[0:H, :])
                    nc.vector.tensor_copy(vT_hi[H:128, sl], ps[H:128, :])
                    if j < 4:
                        psq = ps_proj.tile([H, 512], f32, tag="ps_q")
                        for cc in range(8):
                            nc.tensor.matmul(
                                psq, wq_sb[:, cc, :], xb[:, cc, :],
                                start=(cc == 0), stop=(cc == 7),
                            )
                        nc.vector.tensor_copy(qT[:, sl], psq)
                    # transposes for this block's 4 128-chunks
                    for t in range(4):
                        jj = j * 4 + t
                        c0 = jj * 128
                        ptv = ps_tr.tile([128, H], fp16, tag="ptv")
                        nc.tensor.transpose(ptv, vT_hi[H:128, c0 : c0 + 128], id_hi[H:128, :])
                        nc.vector.tensor_copy(vk[:, jj, 0:H], ptv)
                        ptk = ps_tr.tile([128, H], fp16, tag="ptk")
                        nc.tensor.transpose(ptk, kT[:, c0 : c0 + 128], id_lo)
                        nc.vector.tensor_copy(vk[:, jj, H:128], ptk)

            # ------- merged phases 2+3: scores/softmax/attn + scoresT/out/G --
            # Per q-tile: scores + exp1 for both k-halves, with 4 phase-3
            # j-steps interleaved to keep PE dense (HAM warm) while ACT runs.
            # oG accumulates q-half 0 during q-tiles 0:8, q-half 1 during 8:16.
            with (
                tc.tile_pool(name="attn_t", bufs=3) as attn_pool,
                tc.tile_pool(name="zh", bufs=4) as zh_pool,
                tc.tile_pool(name="est", bufs=3) as est_pool,
                tc.tile_pool(name="ps_s", bufs=2, space="PSUM") as ps_s,
                tc.tile_pool(name="ps_og", bufs=1, space="PSUM") as ps_og,
                tc.tile_pool(name="ps_st", bufs=1, space="PSUM") as ps_st,
            ):
                def p3_step(half, j):
                    st = ps_st.tile([128, 1024], f32, tag="st")
                    for qb in range(2):
                        osl = slice(qb * 512, (qb + 1) * 512)
                        nc.tensor.matmul(
                            st[:, osl],
                            qk[0:H, Q + j * 128 : Q + (j + 1) * 128],
                            qk[0:H, half * 1024 + qb * 512 : half * 1024 + (qb + 1) * 512],
                            start=True, stop=True,
                        )
                    es = est_pool.tile([128, 1024], fp16, tag="es")
                    nc.scalar.activation(out=es, in_=st, func=mybir.ActivationFunctionType.Exp)
                    for qb in range(2):
                        osl = slice(qb * 512, (qb + 1) * 512)
                        nc.tensor.matmul(
                            oG[:, osl],
                            vk[:, j, :],
                            es[:, osl],
                            start=(j == 0), stop=(j == NKT - 1),
                            skip_group_check=True,
                        )

                oG = None
                for i in range(NQT):
                    p3h = i // (NQT // 2)  # oG half for this q-tile
                    if i % (NQT // 2) == 0:
                        oG = ps_og.tile([128, 1024], f32, tag="oG")
                    at = attn_pool.tile([128, T], f32, tag="at")
                    zh = zh_pool.tile([128, 4], f32, tag="zh")
                    for quarter in range(4):
                        ps = ps_s.tile([128, 1024], f32, tag="ps_s")
                        for kb in range(2):
                            c0 = Q + quarter * 1024 + kb * 512
                            nc.tensor.matmul(
                                ps[:, kb * 512 : (kb + 1) * 512],
                                qk[0:H, i * 128 : (i + 1) * 128],
                                qk[0:H, c0 : c0 + 512],
                                start=True, stop=True,
                            )
                        nc.scalar.activation(
                            out=at[:, quarter * 1024 : (quarter + 1) * 1024], in_=ps,
                            func=mybir.ActivationFunctionType.Exp,
                            accum_out=zh[:, quarter : quarter + 1],
                        )
                        if quarter % 2 == 1:
                            for s in range(2):
                                p3_step(p3h, (i % (NQT // 2)) * 4 + (quarter // 2) * 2 + s)
                    nc.vector.tensor_reduce(
                        z_sb[:, i : i + 1], zh,
                        axis=mybir.AxisListType.X, op=mybir.AluOpType.add,
                    )
                    nc.vector.reciprocal(rz_sb[:, i : i + 1], z_sb[:, i : i + 1])
                    nc.vector.tensor_scalar_mul(at, at, rz_sb[:, i : i + 1])
                    nc.sync.dma_start(out=attn_d[i * 128 : (i + 1) * 128, :], in_=at)
                    if i % (NQT // 2) == (NQT // 2) - 1:
                        qsl = slice(p3h * 1024, (p3h + 1) * 1024)
                        nc.vector.tensor_copy(outT_sb[:, qsl], oG[0:H, :])
                        nc.vector.tensor_copy(g_sb[H:128, qsl], oG[H:128, :])

            # ---------------- phase 4: finalize ------------------------------
            nc.sync.dma_start(out=gt_d[:, :], in_=g_sb[H:128, :])
            nc.sync.dma_start(out=qt_d[:, :], in_=qT)
            nc.sync.dma_start(out=zt_d[:, :], in_=z_sb)
            with tc.tile_pool(name="ps_ot", bufs=4, space="PSUM") as ps_ot:
                for jj in range(NQT):
                    po = ps_ot.tile([128, H], f32, tag="po")
                    nc.tensor.transpose(po, outT_sb[:, jj * 128 : (jj + 1) * 128], id_f32)
                    nc.vector.tensor_scalar_mul(outn_sb[:, jj, :], po, rz_sb[:, jj : jj + 1])
            nc.sync.dma_start(out=outp_d.rearrange("(jj p) h -> p jj h", p=128), in_=outn_sb)

    nc.compile()
    return nc


_NC_CACHE = None


def _get_nc():
    global _NC_CACHE
    if _NC_CACHE is None:
        _NC_CACHE = build_nc()
    return _NC_CACHE


def make_in_maps(x, Wq, Wk, Wv):
    id64 = np.eye(H, dtype=np.float32)
    wq_h = np.ascontiguousarray(Wq.T).astype(np.float32) * GAMMA
    wkv_h = np.ascontiguousarray(np.concatenate([Wk.T, Wv.T], axis=1)).astype(np.float32)
    in_maps = []
    xts = {}
    for b in range(B):
        xts[b] = x[b].T.astype(np.float16)  # [C, T] fp16 (device rounds anyway)
    for c in range(N_CORES):
        b, qh = c // 2, c % 2
        xt = xts[b]
        if qh == 1:
            xt = np.ascontiguousarray(
                np.concatenate([xt[:, Q:], xt[:, :Q]], axis=1)
            )
        else:
            xt = np.ascontiguousarray(xt)
        in_maps.append({"xt": xt, "wq": wq_h, "wkv": wkv_h, "id64": id64})
    return in_maps


def gather_outputs(results, energy):
    out = np.empty((B, T, H), dtype=np.float32)
    attn = np.empty((B, T, T), dtype=np.float32)
    ent_sum = 0.0
    for c in range(N_CORES):
        b, qh = c // 2, c % 2
        r = results[c]
        qsl = slice(qh * Q, (qh + 1) * Q)
        out[b, qsl, :] = r["outp"]
        a = r["attn"]
        if qh == 0:
            attn[b, qsl, :] = a
        else:
            attn[b, qsl, Q:] = a[:, :Q]
            attn[b, qsl, :Q] = a[:, Q:]
        Zq = r["zt"].T.reshape(Q).astype(np.float64)  # q = i*128 + p
        P = np.einsum("hq,hq->q", r["qt"].astype(np.float64), r["gt"].astype(np.float64))
        Eq = np.log(Zq) - P / Zq - T * 1e-8
        ent_sum += Eq.sum()
    ent_mean = np.float32(ent_sum / (B * T))
    new_energy = np.float32(0.9 * np.float32(energy) + 0.1 * ent_mean)
    return out, attn, new_energy


def run_on_hw(in_maps, trace=False):
    nc = _get_nc()
    return bass_utils.run_bass_kernel_spmd(
        nc, in_maps, core_ids=list(range(N_CORES)), trace=trace
    )


def kernel(x, Wq, Wk, Wv, energy):
    in_maps = make_in_maps(
        np.asarray(x, dtype=np.float32),
        np.asarray(Wq, dtype=np.float32),
        np.asarray(Wk, dtype=np.float32),
        np.asarray(Wv, dtype=np.float32),
    )
    res = run_on_hw(in_maps, trace=False)
    return gather_outputs(res.results, np.asarray(energy, dtype=np.float32))
